# revision 19
# baseline (speedup 1.0000x reference)
"""Trainium2 Bass kernel for nn_Decoder_32074815767263 (dense_mlp).

Math (per reference):
    enc_proj = enc_state @ W1[:512]          (B,T,H)
    dec_proj = dec_state @ W1[512:]          (B,U,H)
    hidden   = tanh(enc_proj[:,:,None,:] + dec_proj[:,None,:,:] + b1)
    logits   = hidden @ W2 + b2              (B,T,U,V)

Sharding: 8 cores = (B=4) x (U halves of 30). Each core computes its
(300, 30, 1000) output slab independently; no collectives.

Per-core pipeline (SPMD-identical program, data differs per core):
  - input DMA split across both HWDGE rings (SP + ACT) so weights land
    in ~half the serialized time; PE warmup matmuls on a zero tile keep
    the PE HAM at 2.4 GHz through the DMA-bound startup.
  - enc_projT[h,t] / dec_projT[h,u] via bf16 matmuls into fp32 PSUM.
  - hiddenT materialized transposed [H-part, row], row = u*300+t, into a
    4608-column circular SBUF buffer (bf16):
      adds: hid[:,h,span] = encp (bf16) + dec_projT[h,u]+b1 scalar,
            h 0-3 on DVE, h 4-7 on GpSimd
      tanh: ACT, groups of 300/600/768 cols (small first groups unblock
            the first matmul blocks early)
  - PE: per 128-row block, 16 bf16 matmuls (8 H-chunks x 2 vocab halves)
    accumulating into a 2-bank PSUM tile.
  - DVE: single-op drain psum + b2 -> fp32 out tile; stores alternate
    between the SP and ACT HWDGE rings (2:1).
"""

import sys

for _p in ("/opt/trn_rl_repo", "/root/.axon_site/_ro/trn_rl_repo"):
    if _p not in sys.path:
        sys.path.append(_p)

import ml_dtypes
import numpy as np

_B, _T, _U = 4, 300, 60
_D, _H, _V = 512, 1024, 1000
_UC = 30                       # U cols per core
_ROWS = _T * _UC               # 9000 hidden rows per core
_CB = 4608                     # circular hid buffer columns (mult of 128 & 768)
_NBLK = (_ROWS + 127) // 128   # 71 matmul row-blocks
_WARM1 = 16                    # PE warmup matmuls before projections

_PROGRAM = None


def _tanh_boundaries():
    # small groups early (PE has no runway yet; keeps the adds->tanh->block
    # chain fine-grained), 768-col groups at steady state
    bs = list(range(300, 2401, 300))
    while bs[-1] < _ROWS:
        bs.append(min(bs[-1] + 768, _ROWS))
    return bs


def _build_program():
    from contextlib import ExitStack

    import concourse.bass as bass
    import concourse.tile as tile
    from concourse import bacc, mybir

    f32 = mybir.dt.float32
    bf16 = mybir.dt.bfloat16
    f8 = mybir.dt.float8e4
    DR = mybir.MatmulPerfMode.DoubleRow
    Tanh = mybir.ActivationFunctionType.Tanh
    Copy = mybir.ActivationFunctionType.Copy

    nc = bacc.Bacc("TRN2", target_bir_lowering=False, debug=False)

    encT = nc.dram_tensor("encT", [_D, _T], bf16, kind="ExternalInput")
    decT = nc.dram_tensor("decT", [_D, _UC], bf16, kind="ExternalInput")
    W1 = nc.dram_tensor("W1", [2 * _D, _H], bf16, kind="ExternalInput")
    b1T = nc.dram_tensor("b1T", [128, 8], f32, kind="ExternalInput")
    W2 = nc.dram_tensor("W2", [_H - 256, _V], bf16, kind="ExternalInput")
    W28 = nc.dram_tensor("W28", [128, 2, 1024], f8, kind="ExternalInput")
    b2b = nc.dram_tensor("b2b", [128, _V], f32, kind="ExternalInput")
    out = nc.dram_tensor("out", [_T, _UC, _V], f32, kind="ExternalOutput")

    with ExitStack() as ctx:
        tc = ctx.enter_context(tile.TileContext(nc))
        consts = ctx.enter_context(tc.tile_pool(name="consts", bufs=1))
        outp = ctx.enter_context(tc.tile_pool(name="outp", bufs=6))
        psmain = ctx.enter_context(tc.tile_pool(name="psmain", bufs=3, space="PSUM"))
        psproj = ctx.enter_context(tc.tile_pool(name="psproj", bufs=2, space="PSUM"))

        w1t = consts.tile([128, 8, _H], bf16, tag="w1t")
        w2t = consts.tile([128, 6, _V], bf16, tag="w2t")
        w28 = consts.tile([128, 2, 1024], f8, tag="w28")
        encTs = consts.tile([128, 4, _T], bf16, tag="encTs")
        decTs = consts.tile([128, 4, _UC], bf16, tag="decTs")
        b1s = consts.tile([128, 8], f32, tag="b1s")
        b2s = consts.tile([128, _V], f32, tag="b2s")
        encp = consts.tile([128, 8, _T], bf16, tag="encp")
        dpb = consts.tile([128, 8, _UC], f32, tag="dpb")
        hid = consts.tile([128, 8, _CB], bf16, tag="hid")
        hid8 = consts.tile([128, 2, _CB], f8, tag="hid8")
        warm = consts.tile([128, 512], bf16, tag="warm")

        nc.vector.memset(warm[:], 0.0)

        # ---- input DMA: split across the two HWDGE rings, ordered by first use
        W1r = W1[:].rearrange("(c p) h -> p c h", p=128)
        W2r = W2[:].rearrange("(c p) v -> p c v", p=128)
        # SP ring: all W1 (dec chunks first) so both projections unblock early
        nc.sync.dma_start(out=decTs[:], in_=decT[:].rearrange("(c p) t -> p c t", p=128))
        nc.sync.dma_start(out=b1s[:], in_=b1T[:])
        for d in range(4, 8):
            nc.sync.dma_start(out=w1t[:, d, :], in_=W1r[:, d, :])
        # ACT ring: encT + enc W1, then W2 (needed from block 0 on), then b2
        nc.scalar.dma_start(out=encTs[:], in_=encT[:].rearrange("(c p) t -> p c t", p=128))
        for d in range(4):
            nc.scalar.dma_start(out=w1t[:, d, :], in_=W1r[:, d, :])
        nc.scalar.dma_start(out=w28[:], in_=W28[:])
        for c in range(6):
            nc.scalar.dma_start(out=w2t[:, c, :], in_=W2r[:, c, :])
        nc.scalar.dma_start(out=b2s[:], in_=b2b[:])

        # ---- PE warmup: HAM needs ~3.4us of activity to clock up; the
        # startup is DMA-bound, so burn idle PE time on dummy matmuls.
        def emit_warm(n, psd_tile):
            for _ in range(n):
                nc.tensor.matmul(
                    psd_tile[:, 0:500], warm[:, 0:128], warm[:, 0:500],
                    start=True, stop=True,
                )

        dum = psmain.tile([128, 2, 512], f32, tag="ps")
        emit_warm(_WARM1, dum.rearrange("p a n -> p (a n)"))

        # ---- pre-activation adds (DVE only: GpSimd shares the DVE SBUF
        # port and starves it) ----
        def emit_add_one(u, h):
            off = (_T * u) % _CB
            L = min(_T, _CB - off)
            nc.vector.tensor_scalar_add(
                out=hid[:, h, off : off + L],
                in0=encp[:, h, 0:L],
                scalar1=dpb[:, h, u : u + 1],
            )
            if L < _T:
                nc.vector.tensor_scalar_add(
                    out=hid[:, h, 0 : _T - L],
                    in0=encp[:, h, L:_T],
                    scalar1=dpb[:, h, u : u + 1],
                )

        def emit_add(u):
            for h in range(8):
                emit_add_one(u, h)

        # ---- projections (dec first: dpb feeds every add) ----
        psd = psproj.tile([128, 8, 32], f32, tag="pp")
        for h in range(8):
            for d in range(4):
                nc.tensor.matmul(
                    psd[:, h, 0:_UC],
                    w1t[:, 4 + d, 128 * h : 128 * (h + 1)],
                    decTs[:, d, :],
                    start=(d == 0),
                    stop=(d == 3),
                )
        for h in range(8):
            nc.vector.tensor_scalar_add(
                out=dpb[:, h, :], in0=psd[:, h, 0:_UC], scalar1=b1s[:, h : h + 1]
            )
        for h in range(8):
            pse = psproj.tile([128, 304], f32, tag="pp")
            for d in range(4):
                nc.tensor.matmul(
                    pse[:, 0:_T],
                    w1t[:, d, 128 * h : 128 * (h + 1)],
                    encTs[:, d, :],
                    start=(d == 0),
                    stop=(d == 3),
                )
            nc.vector.tensor_copy(out=encp[:, h, :], in_=pse[:, 0:_T])

        # ---- per-block matmul + drain + store ----
        def emit_block(k):
            r0 = 128 * k
            M = min(128, _ROWS - r0)
            c0 = r0 % _CB
            ps = psmain.tile([128, 2, 512], f32, tag="ps")
            for v in range(2):
                for h in range(2, 8):
                    nc.tensor.matmul(
                        ps[:M, v, 0:500],
                        hid[:, h, c0 : c0 + M],
                        w2t[:, h - 2, 500 * v : 500 * (v + 1)],
                        start=(h == 2),
                        stop=False,
                    )
                nc.tensor.matmul(
                    ps[:M, v, 0:500],
                    hid8[:, 0:2, c0 : c0 + M],
                    w28[:, 0:2, 500 * v : 500 * (v + 1)],
                    start=False,
                    stop=True,
                    perf_mode=DR,
                )
            ot = outp.tile([128, _V], f32, tag="ot")
            nc.vector.tensor_add(
                out=ot[:M].rearrange("p (v n) -> p v n", v=2),
                in0=ps[:M, :, 0:500],
                in1=b2s[:M].rearrange("p (v n) -> p v n", v=2),
            )
            ring = nc.sync if k % 3 < 2 else nc.scalar
            u0, t0 = divmod(r0, _T)
            if t0 + M <= _T:
                ring.dma_start(out=out[t0 : t0 + M, u0, :], in_=ot[:M, :])
            else:
                L = _T - t0
                ring.dma_start(out=out[t0:_T, u0, :], in_=ot[0:L, :])
                ring.dma_start(out=out[0 : M - L, u0 + 1, :], in_=ot[L:M, :])

        # ---- main loop: produce u-tiles, tanh groups, consume blocks ----
        bounds = _tanh_boundaries()
        next_blk = 0
        tanh_prev = 0

        def emit_tanh(a, b):
            # in-place bf16 tanh for all chunks; chunks 0,1 additionally get
            # a x0.25 fp8 copy for the DoubleRow matmul (W28 is scaled x4 on
            # the host, keeping the product exact while widening the fp8
            # accuracy margin).
            base = a % _CB
            L = b - a
            spans = (
                [(base, L)] if base + L <= _CB else [(base, _CB - base), (0, L - (_CB - base))]
            )
            for h in range(8):
                for s0, sl in spans:
                    nc.scalar.activation(
                        out=hid[:, h, s0 : s0 + sl],
                        in_=hid[:, h, s0 : s0 + sl],
                        func=Tanh,
                    )
            for h in range(2):
                for s0, sl in spans:
                    nc.scalar.activation(
                        out=hid8[:, h, s0 : s0 + sl],
                        in_=hid[:, h, s0 : s0 + sl],
                        func=Copy,
                        scale=0.25,
                    )

        # Per tanh group: first emit the adds needed TWO groups ahead, so in
        # the DVE queue the adds sit in front of this group's block drains
        # (otherwise a drain waiting on the PE delays the adds -> tanh ->
        # next blocks chain and the PE micro-stalls every few blocks).
        emitted_u = 0
        for gi in range(len(bounds)):
            look = 1 if gi < 8 else 2
            target = bounds[min(gi + look, len(bounds) - 1)]
            need_u = -(-target // _T)
            while emitted_u < min(need_u, _UC):
                emit_add(emitted_u)
                emitted_u += 1
            emit_tanh(tanh_prev, bounds[gi])
            tanh_prev = bounds[gi]
            while next_blk < _NBLK and min(128 * (next_blk + 1), _ROWS) <= tanh_prev:
                emit_block(next_blk)
                next_blk += 1
        assert emitted_u == _UC and next_blk == _NBLK, (emitted_u, next_blk)

    nc.finalize()
    return nc


def _get_program():
    global _PROGRAM
    if _PROGRAM is None:
        _PROGRAM = _build_program()
    return _PROGRAM


def _make_in_maps(enc, dec, W1, b1, W2, b2):
    bf = ml_dtypes.bfloat16
    f8 = ml_dtypes.float8_e4m3
    b1T = np.ascontiguousarray(b1.reshape(8, 128).T)
    b2b = np.ascontiguousarray(np.broadcast_to(b2, (128, _V)))
    W1b = W1.astype(bf)
    W2b = np.ascontiguousarray(W2[256:]).astype(bf)
    W28 = np.zeros((128, 2, 1024), f8)
    for c in range(2):
        W28[:, c, :_V] = (W2[128 * c : 128 * (c + 1), :] * 4.0).astype(f8)
    in_maps = []
    for c in range(8):
        b, half = divmod(c, 2)
        in_maps.append(
            {
                "encT": np.ascontiguousarray(enc[b].T.astype(bf)),
                "decT": np.ascontiguousarray(
                    dec[b, half * _UC : (half + 1) * _UC, :].T.astype(bf)
                ),
                "W1": W1b,
                "b1T": b1T,
                "W2": W2b,
                "W28": W28,
                "b2b": b2b,
            }
        )
    return in_maps


def kernel(enc_state, dec_state, W1, b1, W2, b2):
    from concourse.bass_utils import run_bass_kernel_spmd

    enc = np.ascontiguousarray(np.asarray(enc_state, dtype=np.float32))
    dec = np.ascontiguousarray(np.asarray(dec_state, dtype=np.float32))
    W1 = np.ascontiguousarray(np.asarray(W1, dtype=np.float32))
    b1 = np.ascontiguousarray(np.asarray(b1, dtype=np.float32))
    W2 = np.ascontiguousarray(np.asarray(W2, dtype=np.float32))
    b2 = np.ascontiguousarray(np.asarray(b2, dtype=np.float32))

    nc = _get_program()
    in_maps = _make_in_maps(enc, dec, W1, b1, W2, b2)
    res = run_bass_kernel_spmd(nc, in_maps, list(range(8)))

    full = np.empty((_B, _T, _U, _V), np.float32)
    for c in range(8):
        b, half = divmod(c, 2)
        full[b, :, half * _UC : (half + 1) * _UC, :] = res.results[c]["out"]
    return full


# revision 21
# speedup vs baseline: 1.0062x; 1.0062x over previous
"""Trainium2 Bass kernel for nn_Decoder_32074815767263 (dense_mlp).

Math (per reference):
    enc_proj = enc_state @ W1[:512]          (B,T,H)
    dec_proj = dec_state @ W1[512:]          (B,U,H)
    hidden   = tanh(enc_proj[:,:,None,:] + dec_proj[:,None,:,:] + b1)
    logits   = hidden @ W2 + b2              (B,T,U,V)

Sharding: 8 cores = (B=4) x (U halves of 30). Each core computes its
(300, 30, 1000) output slab independently; no collectives.

Per-core pipeline (SPMD-identical program, data differs per core):
  - input DMA split across both HWDGE rings (SP + ACT) so weights land
    in ~half the serialized time; PE warmup matmuls on a zero tile keep
    the PE HAM at 2.4 GHz through the DMA-bound startup.
  - enc_projT[h,t] / dec_projT[h,u] via bf16 matmuls into fp32 PSUM.
  - hiddenT materialized transposed [H-part, row], row = u*300+t, into a
    4608-column circular SBUF buffer (bf16):
      adds: hid[:,h,span] = encp (bf16) + dec_projT[h,u]+b1 scalar,
            h 0-3 on DVE, h 4-7 on GpSimd
      tanh: ACT, groups of 300/600/768 cols (small first groups unblock
            the first matmul blocks early)
  - PE: per 128-row block, 16 bf16 matmuls (8 H-chunks x 2 vocab halves)
    accumulating into a 2-bank PSUM tile.
  - DVE: single-op drain psum + b2 -> fp32 out tile; stores alternate
    between the SP and ACT HWDGE rings (2:1).
"""

import sys

for _p in ("/opt/trn_rl_repo", "/root/.axon_site/_ro/trn_rl_repo"):
    if _p not in sys.path:
        sys.path.append(_p)

import ml_dtypes
import numpy as np

_B, _T, _U = 4, 300, 60
_D, _H, _V = 512, 1024, 1000
_UC = 30                       # U cols per core
_ROWS = _T * _UC               # 9000 hidden rows per core
_CB = 4608                     # circular hid buffer columns (mult of 128 & 768)
_NBLK = (_ROWS + 127) // 128   # 71 matmul row-blocks
_WARM1 = 22                    # PE warmup matmuls before projections

_PROGRAM = None


def _tanh_boundaries():
    # small groups early (PE has no runway yet; keeps the adds->tanh->block
    # chain fine-grained), 768-col groups at steady state
    bs = list(range(300, 2401, 300))
    while bs[-1] < _ROWS:
        bs.append(min(bs[-1] + 768, _ROWS))
    return bs


def _build_program():
    from contextlib import ExitStack

    import concourse.bass as bass
    import concourse.tile as tile
    from concourse import bacc, mybir

    f32 = mybir.dt.float32
    bf16 = mybir.dt.bfloat16
    f8 = mybir.dt.float8e4
    DR = mybir.MatmulPerfMode.DoubleRow
    Tanh = mybir.ActivationFunctionType.Tanh
    Copy = mybir.ActivationFunctionType.Copy

    nc = bacc.Bacc("TRN2", target_bir_lowering=False, debug=False)

    encT = nc.dram_tensor("encT", [_D, _T], bf16, kind="ExternalInput")
    decT = nc.dram_tensor("decT", [_D, _UC], bf16, kind="ExternalInput")
    W1 = nc.dram_tensor("W1", [2 * _D, _H], bf16, kind="ExternalInput")
    b1T = nc.dram_tensor("b1T", [128, 8], f32, kind="ExternalInput")
    W2 = nc.dram_tensor("W2", [_H - 256, _V], bf16, kind="ExternalInput")
    W28 = nc.dram_tensor("W28", [128, 2, 1024], f8, kind="ExternalInput")
    b2b = nc.dram_tensor("b2b", [128, _V], f32, kind="ExternalInput")
    out = nc.dram_tensor("out", [_T, _UC, _V], f32, kind="ExternalOutput")

    with ExitStack() as ctx:
        tc = ctx.enter_context(tile.TileContext(nc))
        consts = ctx.enter_context(tc.tile_pool(name="consts", bufs=1))
        outp = ctx.enter_context(tc.tile_pool(name="outp", bufs=6))
        psmain = ctx.enter_context(tc.tile_pool(name="psmain", bufs=3, space="PSUM"))
        psproj = ctx.enter_context(tc.tile_pool(name="psproj", bufs=2, space="PSUM"))

        w1t = consts.tile([128, 8, _H], bf16, tag="w1t")
        w2t = consts.tile([128, 6, _V], bf16, tag="w2t")
        w28 = consts.tile([128, 2, 1024], f8, tag="w28")
        encTs = consts.tile([128, 4, _T], bf16, tag="encTs")
        decTs = consts.tile([128, 4, _UC], bf16, tag="decTs")
        b1s = consts.tile([128, 8], f32, tag="b1s")
        b2s = consts.tile([128, _V], f32, tag="b2s")
        encp = consts.tile([128, 8, _T], bf16, tag="encp")
        dpb = consts.tile([128, 8, _UC], f32, tag="dpb")
        hid = consts.tile([128, 8, _CB], bf16, tag="hid")
        hid8 = consts.tile([128, 2, _CB], f8, tag="hid8")
        warm = consts.tile([128, 512], bf16, tag="warm")

        nc.vector.memset(warm[:], 0.0)

        # ---- input DMA: split across the two HWDGE rings, ordered by first use
        W1r = W1[:].rearrange("(c p) h -> p c h", p=128)
        W2r = W2[:].rearrange("(c p) v -> p c v", p=128)
        # SP ring: all W1 (dec chunks first) so both projections unblock early
        nc.sync.dma_start(out=decTs[:], in_=decT[:].rearrange("(c p) t -> p c t", p=128))
        nc.sync.dma_start(out=b1s[:], in_=b1T[:])
        for d in range(4, 8):
            nc.sync.dma_start(out=w1t[:, d, :], in_=W1r[:, d, :])
        # ACT ring: encT + enc W1, then W2 (needed from block 0 on), then b2
        nc.scalar.dma_start(out=encTs[:], in_=encT[:].rearrange("(c p) t -> p c t", p=128))
        for d in range(4):
            nc.scalar.dma_start(out=w1t[:, d, :], in_=W1r[:, d, :])
        nc.scalar.dma_start(out=w28[:], in_=W28[:])
        for c in range(6):
            nc.scalar.dma_start(out=w2t[:, c, :], in_=W2r[:, c, :])
        nc.scalar.dma_start(out=b2s[:], in_=b2b[:])

        # ---- PE warmup: HAM needs ~3.4us of activity to clock up; the
        # startup is DMA-bound, so burn idle PE time on dummy matmuls.
        def emit_warm(n, psd_tile):
            for _ in range(n):
                nc.tensor.matmul(
                    psd_tile[:, 0:500], warm[:, 0:128], warm[:, 0:500],
                    start=True, stop=True,
                )

        dum = psmain.tile([128, 2, 512], f32, tag="ps")
        emit_warm(_WARM1, dum.rearrange("p a n -> p (a n)"))

        # ---- pre-activation adds (DVE only: GpSimd shares the DVE SBUF
        # port and starves it) ----
        def emit_add_one(u, h):
            off = (_T * u) % _CB
            L = min(_T, _CB - off)
            nc.vector.tensor_scalar_add(
                out=hid[:, h, off : off + L],
                in0=encp[:, h, 0:L],
                scalar1=dpb[:, h, u : u + 1],
            )
            if L < _T:
                nc.vector.tensor_scalar_add(
                    out=hid[:, h, 0 : _T - L],
                    in0=encp[:, h, L:_T],
                    scalar1=dpb[:, h, u : u + 1],
                )

        def emit_add(u):
            for h in range(8):
                emit_add_one(u, h)

        # ---- projections (dec first: dpb feeds every add) ----
        psd = psproj.tile([128, 8, 32], f32, tag="pp")
        for h in range(8):
            for d in range(4):
                nc.tensor.matmul(
                    psd[:, h, 0:_UC],
                    w1t[:, 4 + d, 128 * h : 128 * (h + 1)],
                    decTs[:, d, :],
                    start=(d == 0),
                    stop=(d == 3),
                )
        for h in range(8):
            nc.vector.tensor_scalar_add(
                out=dpb[:, h, :], in0=psd[:, h, 0:_UC], scalar1=b1s[:, h : h + 1]
            )
        for h in range(8):
            pse = psproj.tile([128, 304], f32, tag="pp")
            for d in range(4):
                nc.tensor.matmul(
                    pse[:, 0:_T],
                    w1t[:, d, 128 * h : 128 * (h + 1)],
                    encTs[:, d, :],
                    start=(d == 0),
                    stop=(d == 3),
                )
            nc.scalar.activation(
                out=encp[:, h, :], in_=pse[:, 0:_T], func=Copy
            )

        # ---- per-block matmul + drain + store ----
        def emit_block(k):
            r0 = 128 * k
            M = min(128, _ROWS - r0)
            c0 = r0 % _CB
            ps = psmain.tile([128, 2, 512], f32, tag="ps")
            for v in range(2):
                for h in range(2, 8):
                    nc.tensor.matmul(
                        ps[:M, v, 0:500],
                        hid[:, h, c0 : c0 + M],
                        w2t[:, h - 2, 500 * v : 500 * (v + 1)],
                        start=(h == 2),
                        stop=False,
                    )
                nc.tensor.matmul(
                    ps[:M, v, 0:500],
                    hid8[:, 0:2, c0 : c0 + M],
                    w28[:, 0:2, 500 * v : 500 * (v + 1)],
                    start=False,
                    stop=True,
                    perf_mode=DR,
                )
            ot = outp.tile([128, _V], f32, tag="ot")
            nc.vector.tensor_add(
                out=ot[:M].rearrange("p (v n) -> p v n", v=2),
                in0=ps[:M, :, 0:500],
                in1=b2s[:M].rearrange("p (v n) -> p v n", v=2),
            )
            ring = nc.sync if (k < 12 or k % 3 < 2) else nc.scalar
            u0, t0 = divmod(r0, _T)
            if t0 + M <= _T:
                ring.dma_start(out=out[t0 : t0 + M, u0, :], in_=ot[:M, :])
            else:
                L = _T - t0
                ring.dma_start(out=out[t0:_T, u0, :], in_=ot[0:L, :])
                ring.dma_start(out=out[0 : M - L, u0 + 1, :], in_=ot[L:M, :])

        # ---- main loop: produce u-tiles, tanh groups, consume blocks ----
        bounds = _tanh_boundaries()
        next_blk = 0
        tanh_prev = 0

        def emit_tanh(a, b):
            # in-place bf16 tanh for all chunks; chunks 0,1 additionally get
            # a x0.25 fp8 copy for the DoubleRow matmul (W28 is scaled x4 on
            # the host, keeping the product exact while widening the fp8
            # accuracy margin).
            base = a % _CB
            L = b - a
            spans = (
                [(base, L)] if base + L <= _CB else [(base, _CB - base), (0, L - (_CB - base))]
            )
            for h in range(8):
                for s0, sl in spans:
                    nc.scalar.activation(
                        out=hid[:, h, s0 : s0 + sl],
                        in_=hid[:, h, s0 : s0 + sl],
                        func=Tanh,
                    )
            for h in range(2):
                for s0, sl in spans:
                    nc.scalar.activation(
                        out=hid8[:, h, s0 : s0 + sl],
                        in_=hid[:, h, s0 : s0 + sl],
                        func=Copy,
                        scale=0.25,
                    )

        # Per tanh group: first emit the adds needed TWO groups ahead, so in
        # the DVE queue the adds sit in front of this group's block drains
        # (otherwise a drain waiting on the PE delays the adds -> tanh ->
        # next blocks chain and the PE micro-stalls every few blocks).
        emitted_u = 0
        for gi in range(len(bounds)):
            look = 1 if gi < 8 else 2
            target = bounds[min(gi + look, len(bounds) - 1)]
            need_u = -(-target // _T)
            while emitted_u < min(need_u, _UC):
                emit_add(emitted_u)
                emitted_u += 1
            emit_tanh(tanh_prev, bounds[gi])
            tanh_prev = bounds[gi]
            while next_blk < _NBLK and min(128 * (next_blk + 1), _ROWS) <= tanh_prev:
                emit_block(next_blk)
                next_blk += 1
        assert emitted_u == _UC and next_blk == _NBLK, (emitted_u, next_blk)

    nc.finalize()
    return nc


def _get_program():
    global _PROGRAM
    if _PROGRAM is None:
        _PROGRAM = _build_program()
    return _PROGRAM


def _make_in_maps(enc, dec, W1, b1, W2, b2):
    bf = ml_dtypes.bfloat16
    f8 = ml_dtypes.float8_e4m3
    b1T = np.ascontiguousarray(b1.reshape(8, 128).T)
    b2b = np.ascontiguousarray(np.broadcast_to(b2, (128, _V)))
    W1b = W1.astype(bf)
    W2b = np.ascontiguousarray(W2[256:]).astype(bf)
    W28 = np.zeros((128, 2, 1024), f8)
    for c in range(2):
        W28[:, c, :_V] = (W2[128 * c : 128 * (c + 1), :] * 4.0).astype(f8)
    in_maps = []
    for c in range(8):
        b, half = divmod(c, 2)
        in_maps.append(
            {
                "encT": np.ascontiguousarray(enc[b].T.astype(bf)),
                "decT": np.ascontiguousarray(
                    dec[b, half * _UC : (half + 1) * _UC, :].T.astype(bf)
                ),
                "W1": W1b,
                "b1T": b1T,
                "W2": W2b,
                "W28": W28,
                "b2b": b2b,
            }
        )
    return in_maps


def kernel(enc_state, dec_state, W1, b1, W2, b2):
    from concourse.bass_utils import run_bass_kernel_spmd

    enc = np.ascontiguousarray(np.asarray(enc_state, dtype=np.float32))
    dec = np.ascontiguousarray(np.asarray(dec_state, dtype=np.float32))
    W1 = np.ascontiguousarray(np.asarray(W1, dtype=np.float32))
    b1 = np.ascontiguousarray(np.asarray(b1, dtype=np.float32))
    W2 = np.ascontiguousarray(np.asarray(W2, dtype=np.float32))
    b2 = np.ascontiguousarray(np.asarray(b2, dtype=np.float32))

    nc = _get_program()
    in_maps = _make_in_maps(enc, dec, W1, b1, W2, b2)
    res = run_bass_kernel_spmd(nc, in_maps, list(range(8)))

    full = np.empty((_B, _T, _U, _V), np.float32)
    for c in range(8):
        b, half = divmod(c, 2)
        full[b, :, half * _UC : (half + 1) * _UC, :] = res.results[c]["out"]
    return full
